# revision 14
# baseline (speedup 1.0000x reference)
"""LDS (diagonal linear state space + AR) kernel for 8 Trainium2 cores.

Computation (per batch b):
    uB[t, s]   = sum_d x[t, d] * B[d, s]
    h[t]       = A * h[t-1] + uB[t]          (h[-1] = h0, A diagonal)
    lds[t, o]  = sum_s h[t, s] * C[s, o]
    out[t, o]  = sum_{i<10} sum_d M[o, d, i] * x[t-i, d]  +  lds[t+10, o]

Sharding: data-parallel over batch, 2 batches per core, no collectives.

Precision/scales (error budget ~2.7e-3 of max vs 2e-2 gate):
  - uB and C matmuls run in fp8(e4m3) DoubleRow mode (K=256 per
    instruction, 2x fp8 rate). B is scaled by 32, C by 1024 so fp8
    mantissas are used well; the scan state therefore carries 32*h
    (max ~71, fp8 range ~240) and the C-matmul PSUM carries 2^15*lds.
  - AR matmuls run in bf16 with M scaled by 2^15, so the shared PSUM
    accumulation is consistent; one scalar activation with scale 2^-15
    (exact) rescales on the PSUM->SBUF copy.
  - the recurrence is a vector-engine tensor_tensor_scan (fp32 internal
    state) reading uB from PSUM and writing 32*h straight to fp8.

Scheduling: a single interleaved tensor-engine stream produced by a
small co-simulation in _schedule(): uB matmuls are spread through the
stream at the pace the (slower) vector scans consume them, with the
AR/C matmuls of output tiles as filler so the tensor engine never
waits on scans or DMA.
"""

import sys

if "/opt/trn_rl_repo" not in sys.path:
    sys.path.insert(0, "/opt/trn_rl_repo")

import numpy as np
import ml_dtypes

import concourse.bass as bass
import concourse.mybir as mybir
from concourse.tile import TileContext

BSZ = 16
SEQ = 2048
D = 256  # input dim
S = 1024  # state dim
O = 256  # output dim
KX = 10
N_CORES = 8
B_PER_CORE = BSZ // N_CORES  # 2

PAD = 16  # left zero-pad on x time for the AR taps (needs >= KX-1 = 9)
HPAD = 16  # right zero-pad on h time for the +10 shift (needs >= KX)
COL = 1024  # uB/scan column width (one [128, COL] f32 psum tile = 2 banks)
NCOL = SEQ // COL

F32 = mybir.dt.float32
BF16 = mybir.dt.bfloat16
F8 = mybir.dt.float8e4

B_SC = 32.0
C_SC = 1024.0
OUT_INV = 1.0 / (B_SC * C_SC)  # 2^-15, exact

NP_F8 = ml_dtypes.float8_e4m3
NP_BF16 = ml_dtypes.bfloat16

_CACHED = {}

# rough per-op times (us) used only to order the static stream
T_UB = 0.215   # one 512-row DoubleRow matmul
T_SCAN = 2.35  # one [128, 1024] scan on vector (measured)
T_MM = 0.109   # one 256-row matmul
T_ARH = KX * 2 * T_MM
T_CT = 4 * T_MM
T_CP = 0.72    # scalar psum->sbuf copy of one [128, 512] chunk


def _schedule():
    """Static tensor-engine op order via a small co-simulation.

    Emits ("ub", b, c, s, half), ("scan", b, c, s), ("ar", b, j),
    ("c", b, j). Correctness never depends on the estimates (the tile
    framework enforces real deps); this only shapes the issue order.
    """
    # measured: DMA transfers start ~10.2us in (engine preambles) and then
    # move ~1MB/2.5us; cumulative input MB at each tensor's completion:
    #   mbf 1.31 | xbf00 1.84 | xbf01 2.37 | bf8 2.64 | xf8_0 3.17 |
    #   cf8 3.43 | xf8_1 3.96 | xbf10 4.49 | xbf11 5.02
    def arr(m):
        return 9.8 + 2.5 * m
    ub_dma = {0: arr(3.17), 1: arr(3.96)}
    ar_rdy = {0: arr(2.37), 1: arr(5.02)}

    ub_order = [(b, c, s)
                for b in range(B_PER_CORE) for c in range(NCOL)
                for s in range(8)]
    scan_done = {}
    cp_done = []  # completion estimate of each psum chunk's scalar copy
    ops = []
    t = 12.4
    vec_t = 13.0
    gp_t = 13.0
    SEM = 0.55    # semaphore signal latency seen by a waiting matmul
    ub_i = 0
    ub_half = 0
    ar_q = [(b, j) for b in range(B_PER_CORE) for j in range(16)]
    c_q = [(b, j) for b in range(B_PER_CORE) for j in range(16)]
    ar_emitted = set()
    open_pairs = 0  # pairs with AR started but final C not yet emitted

    def c_ready(b, j):
        cn = min(NCOL - 1, (128 * j + KX + 127) // COL)
        return scan_done.get((b, cn, 7), 1e9) + 0.45

    while ub_i < len(ub_order) or ar_q or c_q:
        progress = False
        # 1) uB matmuls: psum chunk n reuses the bank of chunk n-3, which
        #    frees once its scalar copy to SBUF is done -- the scan itself
        #    no longer gates the tensor engine.
        while ub_i < len(ub_order):
            b, c, s = ub_order[ub_i]
            chunk = 2 * ub_i + ub_half
            gate = cp_done[chunk - 3] + SEM if chunk >= 3 else 0.0
            gate = max(gate, ub_dma[b])
            if gate > t + 0.1:
                break
            ops.append(("ub", b, c, s, ub_half))
            t = max(t, gate) + T_UB
            gp_t = max(gp_t, t + 0.05) + T_CP
            cp_done.append(gp_t)
            if ub_half == 1:
                st = max(vec_t, gp_t + 0.05)
                vec_t = st + T_SCAN
                scan_done[(b, c, s)] = vec_t
                ops.append(("scan", b, c, s))
                ub_i += 1
                ub_half = 0
                progress = True
                break
            ub_half = 1
        # 2) filler: prefer a ready C tile, else an AR job
        if c_q and c_q[0] in ar_emitted and c_ready(*c_q[0]) <= t:
            b, j = c_q.pop(0)
            ops.append(("c", b, j))
            t += T_CT
            if j % 2 == 1:
                open_pairs -= 1
            progress = True
        elif ar_q and ar_rdy[ar_q[0][0]] <= t and (
                ar_q[0][1] % 2 == 1 or open_pairs < 5):
            b, j = ar_q.pop(0)
            ops.append(("ar", b, j))
            ar_emitted.add((b, j))
            t += T_ARH
            if j % 2 == 0:
                open_pairs += 1
            progress = True
        elif not progress:
            cand = []
            if c_q and c_q[0] in ar_emitted:
                cand.append(c_ready(*c_q[0]))
            if ar_q:
                cand.append(ar_rdy[ar_q[0][0]])
            if ub_i < len(ub_order):
                chunk = 2 * ub_i + ub_half
                g = cp_done[chunk - 3] + SEM if chunk >= 3 else 0.0
                cand.append(max(g, ub_dma[ub_order[ub_i][0]]))
            t = max(t + 0.05, min(cand) + 0.01) if cand else t + 0.5
    return ops


def _build_nc():
    nc = bass.Bass()

    xf8_d = nc.dram_tensor("xf8", [B_PER_CORE, 128, 2, PAD + SEQ], F8,
                           kind="ExternalInput")
    xbf_d = nc.dram_tensor("xbf", [B_PER_CORE, 2, 128, PAD + SEQ], BF16,
                           kind="ExternalInput")
    bf8_d = nc.dram_tensor("bf8", [128, 2, S], F8, kind="ExternalInput")
    cf8_d = nc.dram_tensor("cf8", [128, 4, 2, O], F8, kind="ExternalInput")
    mbf_d = nc.dram_tensor("mbf", [128, KX, 2, O], BF16,
                           kind="ExternalInput")
    ah_d = nc.dram_tensor("ah", [128, 16], F32, kind="ExternalInput")
    out_d = nc.dram_tensor("out", [B_PER_CORE, SEQ, O], F32,
                           kind="ExternalOutput")

    sched = _schedule()
    HALF = (PAD + SEQ) // 2  # 1032

    with TileContext(nc) as tc:
        with tc.tile_pool(name="persist", bufs=1) as persist, \
             tc.tile_pool(name="osb", bufs=4) as osb_pool, \
             tc.tile_pool(name="ubps", bufs=3, space="PSUM") as ub_pool, \
             tc.tile_pool(name="outps", bufs=5, space="PSUM") as out_pool:

            # ---- persistent operands; DMAs issued in priority order ----
            # AR inputs (mbf, xbf[0]) first: AR jobs are the scan-independent
            # filler that keeps the tensor engine busy from the start, while
            # the vector engine has plenty of slack to start scans later.
            mbf_t = persist.tile([128, KX, 2, O], BF16, tag="mbf", name="mbf")
            mbf = {(i, dch): mbf_t[:, i, dch, :]
                   for i in range(KX) for dch in range(2)}
            xbf = {}
            for b in range(B_PER_CORE):
                for dch in range(2):
                    xbf[b, dch] = persist.tile([128, PAD + SEQ], BF16,
                                               tag=f"xbf{b}{dch}", name=f"xbf{b}{dch}")
            bf8 = persist.tile([128, 2, S], F8, tag="bf8")
            ah = persist.tile([128, 16], F32, tag="ah")
            xf8 = {}
            for b in range(B_PER_CORE):
                xf8[b] = persist.tile([128, 2, PAD + SEQ], F8, tag=f"xf8{b}", name=f"xf8{b}")
            cf8_t = persist.tile([128, 4, 2, O], F8, tag="cf8", name="cf8")
            cf8 = {cc: cf8_t[:, cc] for cc in range(4)}

            # issue input DMAs alternating across the two hwdge queues
            # (sync + scalar) so descriptor issue doesn't serialize
            loads = [
                (mbf_t[:], mbf_d[:]),
                (xbf[0, 0][:], xbf_d[0, 0]),
                (xbf[0, 1][:], xbf_d[0, 1]),
                (bf8[:], bf8_d[:]),
                (ah[:], ah_d[:]),
                (xf8[0][:], xf8_d[0]),
                (cf8_t[:], cf8_d[:]),
                (xf8[1][:], xf8_d[1]),
                (xbf[1, 0][:], xbf_d[1, 0]),
                (xbf[1, 1][:], xbf_d[1, 1]),
            ]
            for k, (dst, srcap) in enumerate(loads):
                eng = nc.sync if k % 2 == 0 else nc.scalar
                eng.dma_start(out=dst, in_=srcap)

            # ---- uB staging in SBUF (bf16), filled by gpsimd copies ----
            ubsb = {}
            for b in range(B_PER_CORE):
                for s in range(8):
                    ubsb[b, s] = persist.tile([128, SEQ], BF16,
                                              tag=f"ub{b}{s}", name=f"ub{b}{s}")

            # ---- h tiles (fp8, DoubleRow k-tile plane layout) ----
            htf8 = {}
            for b in range(B_PER_CORE):
                for cc in range(4):
                    tl = persist.tile([128, 2, SEQ + HPAD], F8,
                                      tag=f"ht{b}{cc}", name=f"ht{b}{cc}")
                    htf8[b, cc] = tl
                    nc.gpsimd.memset(tl[:, :, SEQ:], 0.0)

            # ---- interleaved main stream ----
            pair_ps = {}
            for op in sched:
                kind = op[0]
                if kind == "ub":
                    _, b, c, s, hh = op
                    t0 = c * COL + hh * 512
                    ubp = ub_pool.tile([128, 512], F32, name="ubp")
                    nc.tensor.matmul(
                        out=ubp[:],
                        lhsT=bf8[:, :, s * 128:(s + 1) * 128],
                        rhs=xf8[b][:, :, PAD + t0:PAD + t0 + 512],
                        start=True, stop=True,
                        perf_mode=mybir.MatmulPerfMode.DoubleRow,
                    )
                    nc.scalar.copy(out=ubsb[b, s][:, t0:t0 + 512],
                                   in_=ubp[:])
                elif kind == "scan":
                    _, b, c, s = op
                    ht = htf8[b, s // 2]
                    init = (ah[:, 8 + s:9 + s] if c == 0
                            else ht[:, s % 2, c * COL - 1:c * COL])
                    nc.vector.tensor_tensor_scan(
                        out=ht[:, s % 2, c * COL:(c + 1) * COL],
                        data0=ah[:, s:s + 1].broadcast_to([128, COL]),
                        data1=ubsb[b, s][:, c * COL:(c + 1) * COL],
                        initial=init,
                        op0=mybir.AluOpType.mult,
                        op1=mybir.AluOpType.add,
                    )
                elif kind == "ar":
                    _, b, j = op
                    u = j // 2
                    if j % 2 == 0:
                        pair_ps[b, u] = out_pool.tile([128, 512], F32, name="ops")
                    ops_t = pair_ps[b, u]
                    off = (j % 2) * 256
                    t0 = j * 128
                    # one psum accumulation group per pair (bank): start on
                    # the pair's very first matmul, stop on its very last
                    first = (j % 2 == 0)
                    for i in range(KX):
                        for dch in range(2):
                            nc.tensor.matmul(
                                out=ops_t[:, off:off + 256],
                                lhsT=xbf[b, dch][:, PAD - i + t0:
                                                 PAD - i + t0 + 128],
                                rhs=mbf[i, dch],
                                start=first, stop=False,
                            )
                            first = False
                elif kind == "c":
                    _, b, j = op
                    u = j // 2
                    ops_t = pair_ps[b, u]
                    off = (j % 2) * 256
                    t0 = j * 128
                    for cc in range(4):
                        nc.tensor.matmul(
                            out=ops_t[:, off:off + 256],
                            lhsT=htf8[b, cc][:, :, t0 + KX:t0 + KX + 128],
                            rhs=cf8[cc],
                            start=False, stop=(cc == 3 and j % 2 == 1),
                            perf_mode=mybir.MatmulPerfMode.DoubleRow,
                        )
                    if j % 2 == 1:
                        osb = osb_pool.tile([128, 512], F32, name="osb")
                        nc.scalar.activation(
                            out=osb[:], in_=ops_t[:],
                            func=mybir.ActivationFunctionType.Copy,
                            scale=OUT_INV,
                        )
                        tb = u * 256
                        nc.sync.dma_start(out=out_d[b, tb:tb + 128, :],
                                          in_=osb[:, 0:256])
                        nc.sync.dma_start(out=out_d[b, tb + 128:tb + 256, :],
                                          in_=osb[:, 256:512])

    # Matmult (esp. fused-LDW) supports a limited number of HW sync-wait
    # slots; split excess waits into event-semaphore chains the way
    # Bacc.compile() does.
    import bass_rust as _br
    _br.move_matmul_waits_to_ldweights(nc.m)
    _br.generate_event_semaphores(nc)

    return nc


def _prep_core_inputs(inputs, h0, A, B, C, M, core):
    """Host-side shard + layout/quantization prep for one core."""
    bs = slice(core * B_PER_CORE, (core + 1) * B_PER_CORE)
    x = np.asarray(inputs[bs], np.float32)  # [2, T, D]
    xt = x.transpose(0, 2, 1).reshape(B_PER_CORE, 2, 128, SEQ)  # [b,dch,p,t]

    xf8 = np.zeros((B_PER_CORE, 128, 2, PAD + SEQ), NP_F8)
    xf8[:, :, :, PAD:] = xt.transpose(0, 2, 1, 3).astype(NP_F8)
    xbf = np.zeros((B_PER_CORE, 2, 128, PAD + SEQ), NP_BF16)
    xbf[:, :, :, PAD:] = xt.astype(NP_BF16)

    bf8 = np.ascontiguousarray(
        (B_SC * B).reshape(2, 128, S).transpose(1, 0, 2)).astype(NP_F8)
    cf8 = np.ascontiguousarray(
        (C_SC * C).reshape(4, 2, 128, O).transpose(2, 0, 1, 3)).astype(NP_F8)
    mbf = np.ascontiguousarray(
        (B_SC * C_SC * M).transpose(1, 2, 0).reshape(2, 128, KX, O)
        .transpose(1, 2, 0, 3)).astype(NP_BF16)
    ah = np.zeros((128, 16), np.float32)
    ah[:, :8] = A.reshape(8, 128).T
    ah[:, 8:] = B_SC * h0.reshape(8, 128).T
    return {"xf8": np.ascontiguousarray(xf8),
            "xbf": np.ascontiguousarray(xbf),
            "bf8": bf8, "cf8": cf8, "mbf": mbf, "ah": ah}


LAST_RESULT = None


def kernel(inputs, h0, A, B, C, M):
    global LAST_RESULT
    from concourse.bass_utils import run_bass_kernel_spmd

    inputs = np.asarray(inputs, np.float32)
    h0 = np.asarray(h0, np.float32)
    A = np.asarray(A, np.float32)
    B = np.asarray(B, np.float32)
    C = np.asarray(C, np.float32)
    M = np.asarray(M, np.float32)

    if "nc" not in _CACHED:
        _CACHED["nc"] = _build_nc()
    nc = _CACHED["nc"]

    in_maps = [_prep_core_inputs(inputs, h0, A, B, C, M, c)
               for c in range(N_CORES)]
    res = run_bass_kernel_spmd(nc, in_maps, list(range(N_CORES)))
    LAST_RESULT = res
    out = np.concatenate([res.results[c]["out"] for c in range(N_CORES)],
                         axis=0)
    return out


# revision 15
# speedup vs baseline: 1.0185x; 1.0185x over previous
"""LDS (diagonal linear state space + AR) kernel for 8 Trainium2 cores.

Computation (per batch b):
    uB[t, s]   = sum_d x[t, d] * B[d, s]
    h[t]       = A * h[t-1] + uB[t]          (h[-1] = h0, A diagonal)
    lds[t, o]  = sum_s h[t, s] * C[s, o]
    out[t, o]  = sum_{i<10} sum_d M[o, d, i] * x[t-i, d]  +  lds[t+10, o]

Sharding: data-parallel over batch, 2 batches per core, no collectives.

Precision/scales (error budget ~2.7e-3 of max vs 2e-2 gate):
  - uB and C matmuls run in fp8(e4m3) DoubleRow mode (K=256 per
    instruction, 2x fp8 rate). B is scaled by 32, C by 1024 so fp8
    mantissas are used well; the scan state therefore carries 32*h
    (max ~71, fp8 range ~240) and the C-matmul PSUM carries 2^15*lds.
  - AR matmuls run in bf16 with M scaled by 2^15, so the shared PSUM
    accumulation is consistent; one scalar activation with scale 2^-15
    (exact) rescales on the PSUM->SBUF copy.
  - the recurrence is a vector-engine tensor_tensor_scan (fp32 internal
    state) reading uB from PSUM and writing 32*h straight to fp8.

Scheduling: a single interleaved tensor-engine stream produced by a
small co-simulation in _schedule(): uB matmuls are spread through the
stream at the pace the (slower) vector scans consume them, with the
AR/C matmuls of output tiles as filler so the tensor engine never
waits on scans or DMA.
"""

import sys

if "/opt/trn_rl_repo" not in sys.path:
    sys.path.insert(0, "/opt/trn_rl_repo")

import numpy as np
import ml_dtypes

import concourse.bass as bass
import concourse.mybir as mybir
from concourse.tile import TileContext

BSZ = 16
SEQ = 2048
D = 256  # input dim
S = 1024  # state dim
O = 256  # output dim
KX = 10
N_CORES = 8
B_PER_CORE = BSZ // N_CORES  # 2

PAD = 16  # left zero-pad on x time for the AR taps (needs >= KX-1 = 9)
HPAD = 16  # right zero-pad on h time for the +10 shift (needs >= KX)
COL = 1024  # uB/scan column width (one [128, COL] f32 psum tile = 2 banks)
NCOL = SEQ // COL

F32 = mybir.dt.float32
BF16 = mybir.dt.bfloat16
F8 = mybir.dt.float8e4

B_SC = 32.0
C_SC = 1024.0
OUT_INV = 1.0 / (B_SC * C_SC)  # 2^-15, exact

NP_F8 = ml_dtypes.float8_e4m3
NP_BF16 = ml_dtypes.bfloat16

_CACHED = {}

# rough per-op times (us) used only to order the static stream
T_UB = 0.215   # one 512-row DoubleRow matmul
T_SCAN = 2.35  # one [128, 1024] scan on vector (measured)
T_MM = 0.109   # one 256-row matmul
T_ARH = KX * 2 * T_MM
T_CT = 4 * T_MM
T_CP = 0.72    # scalar psum->sbuf copy of one [128, 512] chunk


def _schedule():
    """Static tensor-engine op order via a small co-simulation.

    Emits ("ub", b, c, s, half), ("scan", b, c, s), ("ar", b, j),
    ("c", b, j). Correctness never depends on the estimates (the tile
    framework enforces real deps); this only shapes the issue order.
    """
    # measured: DMA transfers start ~10.2us in (engine preambles) and then
    # move ~1MB/2.5us; cumulative input MB at each tensor's completion:
    #   mbf 1.31 | xbf00 1.84 | xbf01 2.37 | bf8 2.64 | xf8_0 3.17 |
    #   cf8 3.43 | xf8_1 3.96 | xbf10 4.49 | xbf11 5.02
    def arr(m):
        return 10.2 + 2.5 * m
    ub_dma = {0: arr(3.17), 1: arr(3.96)}
    ar_rdy = {0: arr(2.37), 1: arr(5.02)}

    ub_order = [(b, c, s)
                for b in range(B_PER_CORE) for c in range(NCOL)
                for s in range(8)]
    scan_done = {}
    cp_done = []  # completion estimate of each psum chunk's scalar copy
    ops = []
    t = 12.4
    vec_t = 13.0
    gp_t = 13.0
    SEM = 0.0     # semaphore latency pad (0 = iter4-calibrated best)
    ub_i = 0
    ub_half = 0
    ar_q = [(b, j) for b in range(B_PER_CORE) for j in range(16)]
    c_q = [(b, j) for b in range(B_PER_CORE) for j in range(16)]
    ar_emitted = set()
    open_pairs = 0  # pairs with AR started but final C not yet emitted

    def c_ready(b, j):
        cn = min(NCOL - 1, (128 * j + KX + 127) // COL)
        return scan_done.get((b, cn, 7), 1e9) + 0.25

    while ub_i < len(ub_order) or ar_q or c_q:
        progress = False
        # 1) uB matmuls: psum chunk n reuses the bank of chunk n-3, which
        #    frees once its scalar copy to SBUF is done -- the scan itself
        #    no longer gates the tensor engine.
        while ub_i < len(ub_order):
            b, c, s = ub_order[ub_i]
            chunk = 2 * ub_i + ub_half
            gate = cp_done[chunk - 3] + SEM if chunk >= 3 else 0.0
            gate = max(gate, ub_dma[b])
            if gate > t + 0.1:
                break
            ops.append(("ub", b, c, s, ub_half))
            t = max(t, gate) + T_UB
            gp_t = max(gp_t, t + 0.05) + T_CP
            cp_done.append(gp_t)
            if ub_half == 1:
                st = max(vec_t, gp_t + 0.05)
                vec_t = st + T_SCAN
                scan_done[(b, c, s)] = vec_t
                ops.append(("scan", b, c, s))
                ub_i += 1
                ub_half = 0
                progress = True
                break
            ub_half = 1
        # 2) filler: prefer a ready C tile, else an AR job
        if c_q and c_q[0] in ar_emitted and c_ready(*c_q[0]) <= t:
            b, j = c_q.pop(0)
            ops.append(("c", b, j))
            t += T_CT
            if j % 2 == 1:
                open_pairs -= 1
            progress = True
        elif ar_q and ar_rdy[ar_q[0][0]] <= t and (
                ar_q[0][1] % 2 == 1 or open_pairs < 5):
            b, j = ar_q.pop(0)
            ops.append(("ar", b, j))
            ar_emitted.add((b, j))
            t += T_ARH
            if j % 2 == 0:
                open_pairs += 1
            progress = True
        elif not progress:
            cand = []
            if c_q and c_q[0] in ar_emitted:
                cand.append(c_ready(*c_q[0]))
            if ar_q:
                cand.append(ar_rdy[ar_q[0][0]])
            if ub_i < len(ub_order):
                chunk = 2 * ub_i + ub_half
                g = cp_done[chunk - 3] + SEM if chunk >= 3 else 0.0
                cand.append(max(g, ub_dma[ub_order[ub_i][0]]))
            t = max(t + 0.05, min(cand) + 0.01) if cand else t + 0.5
    return ops


def _build_nc():
    nc = bass.Bass()

    xf8_d = nc.dram_tensor("xf8", [B_PER_CORE, 128, 2, PAD + SEQ], F8,
                           kind="ExternalInput")
    xbf_d = nc.dram_tensor("xbf", [B_PER_CORE, 2, 128, PAD + SEQ], BF16,
                           kind="ExternalInput")
    bf8_d = nc.dram_tensor("bf8", [128, 2, S], F8, kind="ExternalInput")
    cf8_d = nc.dram_tensor("cf8", [128, 4, 2, O], F8, kind="ExternalInput")
    mbf_d = nc.dram_tensor("mbf", [128, KX, 2, O], BF16,
                           kind="ExternalInput")
    ah_d = nc.dram_tensor("ah", [128, 16], F32, kind="ExternalInput")
    out_d = nc.dram_tensor("out", [B_PER_CORE, SEQ, O], F32,
                           kind="ExternalOutput")

    sched = _schedule()
    HALF = (PAD + SEQ) // 2  # 1032

    with TileContext(nc) as tc:
        with tc.tile_pool(name="persist", bufs=1) as persist, \
             tc.tile_pool(name="osb", bufs=4) as osb_pool, \
             tc.tile_pool(name="ubps", bufs=3, space="PSUM") as ub_pool, \
             tc.tile_pool(name="outps", bufs=5, space="PSUM") as out_pool:

            # ---- persistent operands; DMAs issued in priority order ----
            # AR inputs (mbf, xbf[0]) first: AR jobs are the scan-independent
            # filler that keeps the tensor engine busy from the start, while
            # the vector engine has plenty of slack to start scans later.
            mbf_t = persist.tile([128, KX, 2, O], BF16, tag="mbf", name="mbf")
            mbf = {(i, dch): mbf_t[:, i, dch, :]
                   for i in range(KX) for dch in range(2)}
            xbf = {}
            for b in range(B_PER_CORE):
                for dch in range(2):
                    xbf[b, dch] = persist.tile([128, PAD + SEQ], BF16,
                                               tag=f"xbf{b}{dch}", name=f"xbf{b}{dch}")
            bf8 = persist.tile([128, 2, S], F8, tag="bf8")
            ah = persist.tile([128, 16], F32, tag="ah")
            xf8 = {}
            for b in range(B_PER_CORE):
                xf8[b] = persist.tile([128, 2, PAD + SEQ], F8, tag=f"xf8{b}", name=f"xf8{b}")
            cf8_t = persist.tile([128, 4, 2, O], F8, tag="cf8", name="cf8")
            cf8 = {cc: cf8_t[:, cc] for cc in range(4)}

            # issue input DMAs alternating across the two hwdge queues
            # (sync + scalar) so descriptor issue doesn't serialize
            loads = [
                (mbf_t[:], mbf_d[:]),
                (xbf[0, 0][:], xbf_d[0, 0]),
                (xbf[0, 1][:], xbf_d[0, 1]),
                (bf8[:], bf8_d[:]),
                (ah[:], ah_d[:]),
                (xf8[0][:], xf8_d[0]),
                (cf8_t[:], cf8_d[:]),
                (xf8[1][:], xf8_d[1]),
                (xbf[1, 0][:], xbf_d[1, 0]),
                (xbf[1, 1][:], xbf_d[1, 1]),
            ]
            for dst, srcap in loads:
                nc.sync.dma_start(out=dst, in_=srcap)

            # ---- uB staging in SBUF (bf16), filled by gpsimd copies ----
            ubsb = {}
            for b in range(B_PER_CORE):
                for s in range(8):
                    ubsb[b, s] = persist.tile([128, SEQ], BF16,
                                              tag=f"ub{b}{s}", name=f"ub{b}{s}")

            # ---- h tiles (fp8, DoubleRow k-tile plane layout) ----
            htf8 = {}
            for b in range(B_PER_CORE):
                for cc in range(4):
                    tl = persist.tile([128, 2, SEQ + HPAD], F8,
                                      tag=f"ht{b}{cc}", name=f"ht{b}{cc}")
                    htf8[b, cc] = tl
                    nc.gpsimd.memset(tl[:, :, SEQ:], 0.0)

            # ---- interleaved main stream ----
            pair_ps = {}
            for op in sched:
                kind = op[0]
                if kind == "ub":
                    _, b, c, s, hh = op
                    t0 = c * COL + hh * 512
                    ubp = ub_pool.tile([128, 512], F32, name="ubp")
                    nc.tensor.matmul(
                        out=ubp[:],
                        lhsT=bf8[:, :, s * 128:(s + 1) * 128],
                        rhs=xf8[b][:, :, PAD + t0:PAD + t0 + 512],
                        start=True, stop=True,
                        perf_mode=mybir.MatmulPerfMode.DoubleRow,
                    )
                    nc.scalar.copy(out=ubsb[b, s][:, t0:t0 + 512],
                                   in_=ubp[:])
                elif kind == "scan":
                    _, b, c, s = op
                    ht = htf8[b, s // 2]
                    init = (ah[:, 8 + s:9 + s] if c == 0
                            else ht[:, s % 2, c * COL - 1:c * COL])
                    nc.vector.tensor_tensor_scan(
                        out=ht[:, s % 2, c * COL:(c + 1) * COL],
                        data0=ah[:, s:s + 1].broadcast_to([128, COL]),
                        data1=ubsb[b, s][:, c * COL:(c + 1) * COL],
                        initial=init,
                        op0=mybir.AluOpType.mult,
                        op1=mybir.AluOpType.add,
                    )
                elif kind == "ar":
                    _, b, j = op
                    u = j // 2
                    if j % 2 == 0:
                        pair_ps[b, u] = out_pool.tile([128, 512], F32, name="ops")
                    ops_t = pair_ps[b, u]
                    off = (j % 2) * 256
                    t0 = j * 128
                    # one psum accumulation group per pair (bank): start on
                    # the pair's very first matmul, stop on its very last
                    first = (j % 2 == 0)
                    for i in range(KX):
                        for dch in range(2):
                            nc.tensor.matmul(
                                out=ops_t[:, off:off + 256],
                                lhsT=xbf[b, dch][:, PAD - i + t0:
                                                 PAD - i + t0 + 128],
                                rhs=mbf[i, dch],
                                start=first, stop=False,
                            )
                            first = False
                elif kind == "c":
                    _, b, j = op
                    u = j // 2
                    ops_t = pair_ps[b, u]
                    off = (j % 2) * 256
                    t0 = j * 128
                    for cc in range(4):
                        nc.tensor.matmul(
                            out=ops_t[:, off:off + 256],
                            lhsT=htf8[b, cc][:, :, t0 + KX:t0 + KX + 128],
                            rhs=cf8[cc],
                            start=False, stop=(cc == 3 and j % 2 == 1),
                            perf_mode=mybir.MatmulPerfMode.DoubleRow,
                        )
                    if j % 2 == 1:
                        osb = osb_pool.tile([128, 512], F32, name="osb")
                        nc.scalar.activation(
                            out=osb[:], in_=ops_t[:],
                            func=mybir.ActivationFunctionType.Copy,
                            scale=OUT_INV,
                        )
                        tb = u * 256
                        nc.sync.dma_start(out=out_d[b, tb:tb + 128, :],
                                          in_=osb[:, 0:256])
                        nc.sync.dma_start(out=out_d[b, tb + 128:tb + 256, :],
                                          in_=osb[:, 256:512])

    # Matmult (esp. fused-LDW) supports a limited number of HW sync-wait
    # slots; split excess waits into event-semaphore chains the way
    # Bacc.compile() does.
    import bass_rust as _br
    _br.move_matmul_waits_to_ldweights(nc.m)
    _br.generate_event_semaphores(nc)

    return nc


def _prep_core_inputs(inputs, h0, A, B, C, M, core):
    """Host-side shard + layout/quantization prep for one core."""
    bs = slice(core * B_PER_CORE, (core + 1) * B_PER_CORE)
    x = np.asarray(inputs[bs], np.float32)  # [2, T, D]
    xt = x.transpose(0, 2, 1).reshape(B_PER_CORE, 2, 128, SEQ)  # [b,dch,p,t]

    xf8 = np.zeros((B_PER_CORE, 128, 2, PAD + SEQ), NP_F8)
    xf8[:, :, :, PAD:] = xt.transpose(0, 2, 1, 3).astype(NP_F8)
    xbf = np.zeros((B_PER_CORE, 2, 128, PAD + SEQ), NP_BF16)
    xbf[:, :, :, PAD:] = xt.astype(NP_BF16)

    bf8 = np.ascontiguousarray(
        (B_SC * B).reshape(2, 128, S).transpose(1, 0, 2)).astype(NP_F8)
    cf8 = np.ascontiguousarray(
        (C_SC * C).reshape(4, 2, 128, O).transpose(2, 0, 1, 3)).astype(NP_F8)
    mbf = np.ascontiguousarray(
        (B_SC * C_SC * M).transpose(1, 2, 0).reshape(2, 128, KX, O)
        .transpose(1, 2, 0, 3)).astype(NP_BF16)
    ah = np.zeros((128, 16), np.float32)
    ah[:, :8] = A.reshape(8, 128).T
    ah[:, 8:] = B_SC * h0.reshape(8, 128).T
    return {"xf8": np.ascontiguousarray(xf8),
            "xbf": np.ascontiguousarray(xbf),
            "bf8": bf8, "cf8": cf8, "mbf": mbf, "ah": ah}


LAST_RESULT = None


def kernel(inputs, h0, A, B, C, M):
    global LAST_RESULT
    from concourse.bass_utils import run_bass_kernel_spmd

    inputs = np.asarray(inputs, np.float32)
    h0 = np.asarray(h0, np.float32)
    A = np.asarray(A, np.float32)
    B = np.asarray(B, np.float32)
    C = np.asarray(C, np.float32)
    M = np.asarray(M, np.float32)

    if "nc" not in _CACHED:
        _CACHED["nc"] = _build_nc()
    nc = _CACHED["nc"]

    in_maps = [_prep_core_inputs(inputs, h0, A, B, C, M, c)
               for c in range(N_CORES)]
    res = run_bass_kernel_spmd(nc, in_maps, list(range(N_CORES)))
    LAST_RESULT = res
    out = np.concatenate([res.results[c]["out"] for c in range(N_CORES)],
                         axis=0)
    return out
